# revision 1
# baseline (speedup 1.0000x reference)
"""CIN forward kernel for Trainium2, 8 cores (pipelined, Pool-offloaded,
tapered startup blocks).

Reference computation (per batch b, per position d):
  h0 = x                                  # [F=64, D=64] fields
  for layer l in (1, 2):
    z[(i,j), n] = x[i, n] * h[j, n]       # outer product, K = F*F = 4096
    h[o, n] = relu(sum_c W_l[o, c] z[c, n] + b_l[o])
  y[b] = sum_d concat(h1, h2)[:, b, :] @ wfc + bfc

Distribution: pure data parallel — batch (1024) split 128/core over 8 cores;
weights replicated; no collectives.

Per-core schedule (n = 8192 cols processed in column blocks; the first blocks
are 256 wide so the DMA pipeline-fill debt is smaller, the rest 512):
  - z build split across engines: DVE ~26 chunks/layer (tensor_tensor at
    2x fp16 rate), Pool (gpsimd) ~6 (tensor_tensor at 0.42 eff) — the z wall
    drops from 273us (DVE-only) to ~220us.
  - W1 stored with duplicated M=128 columns: matmul cost is N-cycles
    regardless of M, so the idle array half produces the h1
    partition-duplicate for free (no cross-partition DMA in the handoff).
  - PE order L1(k+1), L2(k): the PE streams block k+1's first layer during
    block k's epilogue handoff, so it never idles.
  - DVE order per period: z1(k+2) piece0, z2(k) x3, z1(k+2) p1/p2.
  - d-pooling runs as ACT accumulate-copies (8 per layer-block) into a
    persistent pooled tile; the FC collapses to ONE K=128 matmul at the end.
    The last block's pooling/epilogue runs on DVE (idle at drain).
  - X broadcast operand is host-prereplicated (xrep, 64MB/core — the
    information-theoretic minimum: every partition needs its own copy of 32
    x-rows) and streamed per-block in 4 pieces.
"""

import numpy as np

import concourse.bacc as bacc
import concourse.mybir as mybir
import concourse.tile as tile
from concourse.bass_utils import run_bass_kernel_spmd

F = 64          # fields
D = 64          # embedding dim
B = 1024        # full batch
NCORES = 8
B_LOC = B // NCORES          # 128 batches per core
NCH = (F * F) // 128         # 32 contraction chunks of 128
MACRO = 512                  # xrep DRAM blocking width
N_TOTAL = B_LOC * D          # 8192
f16 = mybir.dt.float16
f32 = mybir.dt.float32

# column blocks (n0, width): tapered start primes the pipeline faster
BLOCKS = [(i * 256, 256) for i in range(8)] + [(2048 + i * 512, 512) for i in range(9)] + [(6656 + i * 256, 256) for i in range(6)]
NB = len(BLOCKS)
assert sum(w for _, w in BLOCKS) == N_TOTAL


# z-chunk split: DVE pieces (start, nchunks), Pool piece, alternating parity
def _pieces(k):
    if k % 2 == 0:
        return [(0, 9), (9, 9), (18, 8)], (26, 6)
    return [(0, 9), (9, 8), (17, 8)], (25, 7)


def build_cin_nc():
    nc = bacc.Bacc(None)

    xt = nc.dram_tensor("xt16", [F, N_TOTAL], f16, kind="ExternalInput")
    xrep = nc.dram_tensor(
        "xrep", [(N_TOTAL // MACRO) * 128, NCH * MACRO], f16,
        kind="ExternalInput",
    )
    w1d = nc.dram_tensor("w1sb", [128, NCH * 128], f16, kind="ExternalInput")
    w2d = nc.dram_tensor("w2sb", [128, NCH * F], f16, kind="ExternalInput")
    b1d = nc.dram_tensor("b1dup", [128, 1], f32, kind="ExternalInput")
    b2d = nc.dram_tensor("b2dup", [128, 1], f32, kind="ExternalInput")
    wfcd = nc.dram_tensor("wfccat", [128, 1], f32, kind="ExternalInput")
    bfcd = nc.dram_tensor("bfc", [1, 1], f32, kind="ExternalInput")
    yd = nc.dram_tensor("y", [1, B_LOC], f32, kind="ExternalOutput")

    mult = mybir.AluOpType.mult
    Relu = mybir.ActivationFunctionType.Relu
    Copy = mybir.ActivationFunctionType.Copy

    with tile.TileContext(nc) as tc:
        with (
            tc.tile_pool(name="const", bufs=1) as cpool,
            tc.tile_pool(name="xbig", bufs=3) as xpool,
            tc.tile_pool(name="xa", bufs=4) as xapool,
            tc.tile_pool(name="xd", bufs=3) as xdpool,
            tc.tile_pool(name="zd", bufs=6) as zdpool,
            tc.tile_pool(name="zp", bufs=3) as zppool,
            tc.tile_pool(name="h", bufs=3) as hpool,
            tc.tile_pool(name="psum", bufs=4, space="PSUM") as ppool,
            tc.tile_pool(name="psumfc", bufs=1, space="PSUM") as fcpool,
        ):
            w1 = cpool.tile([128, NCH * 128], f16)
            w2 = cpool.tile([128, NCH * F], f16)
            b1 = cpool.tile([128, 1], f32)
            b2 = cpool.tile([128, 1], f32)
            wfc = cpool.tile([128, 1], f32)
            bfc = cpool.tile([1, 1], f32)
            pooled = cpool.tile([128, B_LOC], f32)   # [0:64]=L1, [64:128]=L2
            y_sb = cpool.tile([1, B_LOC], f32)
            scr = cpool.tile([128, D], f16)          # discard target for accums

            Xs = {}     # k -> dict piece -> (tile, c0, ncch)
            xds = {}    # k -> xdup tile
            Hps = {}    # k -> h1 dup tile
            h2s = {}    # k -> h2 tile (valid on [64:128])
            z1s = {}
            z2s = {}
            ps1 = {}
            ps2 = {}

            # slot sizes (columns of chunks at full width)
            X_ALLOC = {"A": 9, "B": 9, "C": 8, "P": 7}

            def load_x_piece(k, piece):
                n0, w = BLOCKS[k]
                dve_p, pool_p = _pieces(k)
                c0, ncch = {"A": dve_p[0], "B": dve_p[1], "C": dve_p[2],
                            "P": pool_p}[piece]
                pool_sel = xapool if piece == "A" else xpool
                t = pool_sel.tile(
                    [128, X_ALLOC[piece] * MACRO], f16, tag="X" + piece
                )
                rb = (n0 // MACRO) * 128
                sub = n0 % MACRO
                src = xrep[rb : rb + 128, :].rearrange(
                    "p (c n) -> p c n", n=MACRO
                )[:, c0 : c0 + ncch, sub : sub + w]
                nc.sync.dma_start(
                    out=t[:, 0 : ncch * w].rearrange("p (c n) -> p c n", n=w),
                    in_=src,
                )
                Xs.setdefault(k, {})[piece] = (t, c0, ncch)

            def load_xd(k):
                n0, w = BLOCKS[k]
                xd = xdpool.tile([128, MACRO], f16, tag="xd")
                nc.scalar.dma_start(out=xd[0:64, 0:w], in_=xt[:, n0 : n0 + w])
                nc.scalar.dma_start(out=xd[64:128, 0:w], in_=xt[:, n0 : n0 + w])
                xds[k] = xd

            def z_op(eng, pool_, tag, alloc_ch, w, hdup, X, c0, ncch):
                zfull = pool_.tile([128, alloc_ch * MACRO], f16, tag=tag)
                z = zfull[:, 0 : ncch * w]
                nc_eng = nc.vector if eng == "dve" else nc.gpsimd
                nc_eng.tensor_tensor(
                    z.rearrange("p (f n) -> p f n", n=w),
                    hdup.unsqueeze(1).broadcast_to([128, ncch, w]),
                    X.rearrange("p (f n) -> p f n", n=w),
                    mult,
                )
                return (zfull, c0, ncch)

            def z_dve_piece(k, layer, idx, split=False):
                n0, w = BLOCKS[k]
                hdup = xds[k][:, 0:w] if layer == 1 else Hps[k][:, 0:w]
                c0, ncch = _pieces(k)[0][idx]
                Xt, xc0, xnc = Xs[k]["ABC"[idx]]
                assert xc0 == c0 and xnc == ncch
                zl = z1s if layer == 1 else z2s
                if not split:
                    zl.setdefault(k, {})[c0] = z_op(
                        "dve", zdpool, "zD", 9, w, hdup,
                        Xt[:, 0 : ncch * w], c0, ncch,
                    )
                    return
                h1 = ncch // 2
                zl.setdefault(k, {})[c0] = z_op(
                    "dve", zdpool, "zD", 9, w, hdup,
                    Xt[:, 0 : h1 * w], c0, h1,
                )
                zl[k][c0 + h1] = z_op(
                    "dve", zdpool, "zD", 9, w, hdup,
                    Xt[:, h1 * w : ncch * w], c0 + h1, ncch - h1,
                )

            def z_pool_piece(k, layer):
                n0, w = BLOCKS[k]
                hdup = xds[k][:, 0:w] if layer == 1 else Hps[k][:, 0:w]
                c0, ncch = _pieces(k)[1]
                Xt, xc0, xnc = Xs[k]["P"]
                zl = z1s if layer == 1 else z2s
                zl.setdefault(k, {})[c0] = z_op(
                    "pool", zppool, "zP", 7, w, hdup,
                    Xt[:, 0 : ncch * w], c0, ncch,
                )

            def pe_layer(k, layer):
                n0, w = BLOCKS[k]
                wgt = w1 if layer == 1 else w2
                mw = 128 if layer == 1 else F
                zl = (z1s if layer == 1 else z2s)[k]
                ps = ppool.tile([128, MACRO], f32, tag="ps")
                lo, hi = (0, 128) if layer == 1 else (64, 128)
                for c in range(NCH):
                    for c0 in zl:
                        zt, zc0, znc = zl[c0]
                        if zc0 <= c < zc0 + znc:
                            off = (c - zc0) * w
                            break
                    nc.tensor.matmul(
                        ps[lo:hi, 0:w], wgt[:, c * mw : (c + 1) * mw],
                        zt[:, off : off + w],
                        start=(c == 0), stop=(c == NCH - 1),
                    )
                if layer == 1:
                    ps1[k] = ps
                else:
                    ps2[k] = ps
                del (z1s if layer == 1 else z2s)[k]

            def epi1(k):
                n0, w = BLOCKS[k]
                Hp = hpool.tile([128, MACRO], f16, tag="Hp")
                nc.scalar.activation(
                    Hp[:, 0:w], ps1[k][:, 0:w], Relu, bias=b1[:, :]
                )
                Hps[k] = Hp
                del ps1[k]

            def epi2(k):
                n0, w = BLOCKS[k]
                h2 = hpool.tile([128, MACRO], f16, tag="h2")
                if k == NB - 1:
                    # tail: bias+relu on DVE (idle) to skip the ACT hop
                    nc.vector.tensor_scalar(
                        h2[64:128, 0:w], ps2[k][64:128, 0:w], b2[64:128, :],
                        0.0, mybir.AluOpType.add, mybir.AluOpType.max,
                    )
                else:
                    nc.scalar.activation(
                        h2[64:128, 0:w], ps2[k][64:128, 0:w], Relu,
                        bias=b2[64:128, :],
                    )
                h2s[k] = h2
                del ps2[k]

            def red1(k):
                n0, w = BLOCKS[k]
                nb = w // D
                cb = n0 // D
                for j in range(nb):
                    nc.scalar.activation(
                        scr[0:64, :], Hps[k][0:64, j * D : (j + 1) * D], Copy,
                        accum_out=pooled[0:64, cb + j : cb + j + 1],
                    )

            def red2(k):
                n0, w = BLOCKS[k]
                nb = w // D
                cb = n0 // D
                if k == NB - 1:
                    # tail: ACT accums would delay the final FC; DVE is idle
                    nc.vector.tensor_reduce(
                        pooled[64:128, cb : cb + nb],
                        h2s[k][64:128, 0:w].rearrange("p (b d) -> p b d", d=D),
                        mybir.AxisListType.X, mybir.AluOpType.add,
                    )
                else:
                    for j in range(nb):
                        nc.scalar.activation(
                            scr[64:128, :],
                            h2s[k][64:128, j * D : (j + 1) * D], Copy,
                            accum_out=pooled[64:128, cb + j : cb + j + 1],
                        )
                del h2s[k]

            # ---------------- prologue ----------------
            load_xd(0)
            load_x_piece(0, "A")
            load_x_piece(0, "P")
            load_x_piece(0, "B")
            load_x_piece(0, "C")
            nc.scalar.dma_start(out=w1[:], in_=w1d[:])
            z_dve_piece(0, 1, 0)
            z_pool_piece(0, 1)
            load_xd(1)
            load_x_piece(1, "A")
            load_x_piece(1, "P")
            nc.scalar.dma_start(out=b1[:], in_=b1d[:])
            load_x_piece(1, "B")
            load_x_piece(1, "C")
            nc.scalar.dma_start(out=w2[:], in_=w2d[:])
            nc.scalar.dma_start(out=b2[:], in_=b2d[:])
            nc.scalar.dma_start(out=wfc[:], in_=wfcd[:])
            nc.scalar.dma_start(out=bfc[:], in_=bfcd[:])
            z_dve_piece(0, 1, 1)
            z_dve_piece(0, 1, 2)
            pe_layer(0, 1)
            epi1(0)
            load_xd(2)
            for p in "APBC":
                load_x_piece(2, p)
            z_dve_piece(1, 1, 0)
            z_pool_piece(1, 1)
            z_dve_piece(1, 1, 1)
            z_dve_piece(1, 1, 2)

            # ---------------- steady state ----------------
            for k in range(NB):
                early = k < 10
                if k + 2 < NB:
                    load_xd(k + 2)
                    load_x_piece(k + 2, "A")
                    if not early:
                        z_dve_piece(k + 2, 1, 0)
                if k + 1 < NB:
                    pe_layer(k + 1, 1)
                    epi1(k + 1)
                z_pool_piece(k, 2)
                z_dve_piece(k, 2, 0)
                z_dve_piece(k, 2, 1)
                z_dve_piece(k, 2, 2, split=(k == NB - 1))
                if k + 2 < NB:
                    load_x_piece(k + 2, "P")
                    load_x_piece(k + 2, "B")
                    load_x_piece(k + 2, "C")
                    if early:
                        z_dve_piece(k + 2, 1, 0)
                    z_pool_piece(k + 2, 1)
                    z_dve_piece(k + 2, 1, 1)
                    z_dve_piece(k + 2, 1, 2)
                pe_layer(k, 2)
                epi2(k)
                if k + 1 < NB:
                    red1(k + 1)
                if k == 0:
                    red1(0)
                red2(k)

            # ---------------- FC ----------------
            yp = fcpool.tile([1, B_LOC], f32)
            nc.tensor.matmul(yp[:], wfc[:], pooled[:], start=True, stop=True)
            nc.vector.tensor_scalar_add(y_sb[:], yp[:], bfc[:])
            nc.sync.dma_start(out=yd[:], in_=y_sb[:])

    return nc


def _prep_shared(W1, b1, W2, b2, Wfc, bfc):
    """Host-side weight relayout (replicated on every core)."""
    def lay_w(W, dup=False):
        # w[p, c*M + o] = W[o mod 64, c*128 + p]
        wt = np.ascontiguousarray(W.T.astype(np.float16))      # [4096, 64]
        w = wt.reshape(NCH, 128, F)
        if dup:
            w = np.concatenate([w, w], axis=2)                 # M = 128
        mw = w.shape[2]
        return np.ascontiguousarray(
            w.transpose(1, 0, 2).reshape(128, NCH * mw)
        )

    return {
        "w1sb": lay_w(W1, dup=True),
        "w2sb": lay_w(W2),
        "b1dup": np.concatenate([b1, b1]).reshape(128, 1).astype(np.float32),
        "b2dup": np.concatenate([b2, b2]).reshape(128, 1).astype(np.float32),
        "wfccat": Wfc.reshape(128, 1).astype(np.float32),
        "bfc": bfc.reshape(1, 1).astype(np.float32),
    }


def _prep_x(xt):
    """Per-macro-blocked replicated X layout.
    xrep[m*128 + p, c*MACRO + nn] = xt[2c + p//64, m*MACRO + nn]"""
    n_macro = N_TOTAL // MACRO
    xm = xt.reshape(F, n_macro, MACRO)                   # [row, m, nn]
    sel = xm.reshape(NCH, 2, n_macro, MACRO)             # [c, r, m, nn]
    rep = np.broadcast_to(
        sel.transpose(2, 1, 0, 3)[:, :, None, :, :],     # [m, r, 1, c, nn]
        (n_macro, 2, 64, NCH, MACRO),
    )
    return np.ascontiguousarray(rep).reshape(n_macro * 128, NCH * MACRO)


_NC_CACHE = {}


def _get_nc():
    if "nc" not in _NC_CACHE:
        nc = build_cin_nc()
        nc.finalize()
        _NC_CACHE["nc"] = nc
    return _NC_CACHE["nc"]


def run(x, W1, b1, W2, b2, Wfc, bfc, trace=False, **spmd_kwargs):
    x = np.asarray(x, dtype=np.float32)
    shared = _prep_shared(
        np.asarray(W1, np.float32), np.asarray(b1, np.float32),
        np.asarray(W2, np.float32), np.asarray(b2, np.float32),
        np.asarray(Wfc, np.float32), np.asarray(bfc, np.float32),
    )
    in_maps = []
    for c in range(NCORES):
        xc = x[c * B_LOC : (c + 1) * B_LOC]                    # [128, F, D]
        xtc = np.ascontiguousarray(
            xc.transpose(1, 0, 2).reshape(F, B_LOC * D).astype(np.float16)
        )
        in_maps.append({"xt16": xtc, "xrep": _prep_x(xtc), **shared})
    nc = _get_nc()
    res = run_bass_kernel_spmd(
        nc, in_maps, list(range(NCORES)), trace=trace, **spmd_kwargs
    )
    ys = [np.asarray(res.results[i]["y"]).reshape(B_LOC) for i in range(NCORES)]
    out = np.concatenate(ys).reshape(B, 1).astype(np.float32)
    return out, res


def kernel(x, W1, b1, W2, b2, Wfc, bfc):
    out, _ = run(x, W1, b1, W2, b2, Wfc, bfc, trace=False)
    return out

